# revision 52
# baseline (speedup 1.0000x reference)
"""Trainium2 Bass kernel for nn_DenoisingModule (non-local attention block).

Reference computation (per batch element n, with C=256 channels, HW=4096):
    theta = W_t x + b_t            # queries  [C, HW]
    phi   = W_p x + b_p            # keys     [C, HW]
    g     = x                      # values   [C, HW]
    S     = theta^T phi / sqrt(C)  # [HW, HW]
    A     = softmax(S, axis=keys)
    f     = g A^T                  # [C, HW]
    out   = x + W_c f + b_c

Sharding: 8 cores; each of the N=4 batch elements is split across 2 cores
by query position (2048 queries per core). Every core holds the full key
set for its batch element, so no collectives are needed.

Per-core device program (SPMD, identical on all cores, data differs):
  - scores are computed TRANSPOSED (S^T[q, p] = phi^T theta) so that the
    softmax key-reduction lands on the PSUM partition axis, which lets the
    exp output E^T[q, p] feed the PV matmul directly (no PE transposes).
  - the attention matmuls (scores and PV) run in fp8e4 DoubleRow mode:
    the PE virtualizes to 128x256, doing the full 256-deep contraction in
    one instruction at 2 MACs/cell/cycle. theta/phi are written to fp8 by
    the projection eviction; exp outputs are fp8 with a constant -2 bias
    folded into the activation (softmax is shift-invariant) to keep
    exp values inside fp8e4 range.
  - softmax denominators accumulate on the PE as ones-row DoubleRow
    matmuls into a [1, 512] PSUM slot (one per exp pair, accumulating
    across the whole query group); 1/Sum comes from the DVE
    reciprocal_approx_fast custom op, so the Scalar engine runs ONLY exp
    and never swaps activation tables.
  - residual + bias are fused into the final PSUM eviction; the residual
    is read from the fp16 xk tile (no separate fp32 xq load).
  - the attention loop is software-pipelined (PV trails scores/exp by one
    pair; per-group normalize/conv work is deferred into the next group).

Toolchain constraint that shapes this file: every TPB engine instruction
(and every DMA) may carry at most ONE semaphore wait, so cross-engine
fan-in is funneled through per-engine collector chains, persistent ring
tiles replace rotating tile pools, and loads/stores are merged so each
DMA is the first instruction on its hardware queue.

The host wrapper rolls x columns per-core so queries are always columns
[0, P) of the local key matrix (keeps the program identical across cores),
and pre-transposes x (and the weight matrices) since the PV matmul needs
x^T as the stationary operand.
"""

import numpy as np

import concourse.bass as bass
import concourse.mybir as mybir
from concourse import bacc
from concourse.bass_utils import run_bass_kernel_spmd
from concourse.tile import TileContext, add_dep_helper

N, C, H, W = 4, 256, 64, 64
HW = H * W
NCORES = 8
CORES_PER_N = NCORES // N
P_CORE = HW // CORES_PER_N  # queries per core

F32 = mybir.dt.float32
F32R = mybir.dt.float32r
FP8 = mybir.dt.float8e4
DR = mybir.MatmulPerfMode.DoubleRow

EBIAS = -4.0  # constant shift inside exp (softmax-invariant); keeps
              # exp values in fp8e4 range (<= 240): max observed
              # score*scale is ~7.5 -> e^3.5 = 33, margin up to score 9.4.
              # Keys with score < -2.2 flush to zero weight (< 1e-3 of the
              # softmax mass at this score distribution).


def build_program(P, Q, Cc=C, mm_dt=mybir.dt.float16):
    """Build the per-core Bass program.

    P: queries handled by this core (first P columns of xk)
    Q: total key positions
    mm_dt: dtype for the projection/out-conv matmuls (fp16); the
      attention matmuls are always fp8e4 DoubleRow.
    """
    assert P % 512 == 0 and Q % 512 == 0 and Cc == 256
    CT = Cc // 128
    QT = Q // 128
    PG = P // 512
    QG = Q // 512
    NP = QT // 2  # exp pairs per query group
    # the host pre-scales the theta/phi weights x16 each, so raw scores
    # come out 256x larger; fold the correction into the exp scale
    scale = float(Cc) ** -0.5 / 256.0

    NCH = Q // 512  # xk arrives in 512-column chunks (= projection groups)

    nc = bacc.Bacc("TRN2", target_bir_lowering=False)
    # All inputs arrive pre-interleaved from the host (partition-major) so
    # every DMA descriptor is a 1-8KB contiguous run.  x and the
    # theta/phi weights are fp8 (DoubleRow projections); the residual
    # copy xq and the out-conv weight stay fp16.  The theta/phi weights
    # are pre-scaled x16 on the host (folded back out via the exp scale)
    # so their fp8 encoding uses the full normal range.
    xk = nc.declare_dram_parameter("xk", [128, NCH, CT, 512], FP8, isOutput=False)[:]
    xt = nc.declare_dram_parameter("xt", [128, Q // 128, Cc], FP8, isOutput=False)[:]
    wqk = nc.declare_dram_parameter("wqk", [128, 2, CT, Cc], FP8, isOutput=False)[:]
    wc = nc.declare_dram_parameter("wc", [128, CT, Cc], mm_dt, isOutput=False)[:]
    xq = nc.declare_dram_parameter("xq", [128, CT, P], mm_dt, isOutput=False)[:]
    bcat = nc.declare_dram_parameter("bcat", [3, Cc], F32, isOutput=False)[:]
    # fp16 output (upcast on host): halves the store traffic in the tail
    out = nc.declare_dram_parameter("out", [Cc, P], mm_dt, isOutput=True)[:]

    add = mybir.AluOpType.add

    with TileContext(nc) as tc:
        with (
            tc.tile_pool(name="const", bufs=1) as const,
            tc.tile_pool(name="big", bufs=1) as big,
            tc.tile_pool(name="pss", bufs=1, space="PSUM") as pss,
            tc.tile_pool(name="psf", bufs=1, space="PSUM") as psf,
            tc.tile_pool(name="pso", bufs=1, space="PSUM") as pso,
        ):
            # ---- input loads; xk is split into 512-col chunks so the
            # projections (and behind them the attention pipeline) start
            # as soon as the first chunk lands.  Late-needed tensors (xt,
            # wc, xq) are queued last so they don't contend with the
            # chunks that gate the pipeline start. ----
            ws_sb = const.tile([128, 2, CT, Cc], FP8, tag="ws")
            w_load = nc.sync.dma_start(out=ws_sb, in_=wqk)
            xk_sb = big.tile([128, NCH, CT, 512], FP8, tag="xk")
            xt_sb = big.tile([128, QT, Cc], FP8, tag="xt")
            xk_loads = [
                nc.sync.dma_start(out=xk_sb[:, h], in_=xk[:, h])
                for h in range(NCH)
            ]
            xt_loads = [
                nc.sync.dma_start(
                    out=xt_sb[:, h * (QT // 2) : (h + 1) * (QT // 2)],
                    in_=xt[:, h * (QT // 2) : (h + 1) * (QT // 2)],
                )
                for h in range(2)
            ]
            bb = const.tile([128, 3, CT], F32, tag="bb")
            b_load = nc.sync.dma_start(
                out=bb, in_=bcat.rearrange("w (a p) -> p w a", p=128)
            )
            wc_sb = const.tile([128, CT, Cc], mm_dt, tag="wc")
            wc_load = nc.sync.dma_start(out=wc_sb, in_=wc)
            xq_sb = big.tile([128, CT, P], mm_dt, tag="xq")
            xq_load = nc.sync.dma_start(out=xq_sb, in_=xq)

            # persistent tiles (deliberately NOT pool-rotated: pool-slot
            # releases fan in multiple procs; rings keep wait fan-in low)
            th_sb = big.tile([128, CT, P], FP8, tag="th")
            ph_sb = big.tile([128, CT, Q], FP8, tag="ph")
            f_sb = big.tile([128, CT, P], mm_dt, tag="f")
            e_ring = big.tile([128, 4, 2, 512], FP8, tag="ering")
            rc_ring = const.tile([1, PG, 512], F32R, tag="rcring")
            bc_ring = big.tile([128, PG, 512], F32, tag="bcring")
            o_ring = big.tile([128, CT, PG, 512], mm_dt, tag="oring")

            # ---- engine program-order chains + wait collectors ----
            last = {}

            def chain(eng, inst):
                # ordering edges disabled: Bacc legalizes multi-waits, so the
                # Tile scheduler is free to interleave within each engine
                last[eng] = inst.ins
                return inst

            ones_f = const.tile([128, 1], F32, tag="ones_f")
            chain("v", nc.vector.memset(ones_f, 1.0))
            ones_col = const.tile([1, 128], F32R, tag="ones_col")
            chain("v", nc.vector.tensor_copy(
                ones_col, ones_f[0:1, 0:1].to_broadcast([1, 128])))
            ones8 = const.tile([128, 2, 16], FP8, tag="ones8")
            chain("v", nc.vector.memset(ones8, 1.0))
            zbias = const.tile([128, 1], F32, tag="zbias")
            chain("v", nc.vector.memset(zbias, 0.0))
            ebias = const.tile([128, 1], F32, tag="ebias")
            eb_inst = chain("v", nc.vector.memset(ebias, EBIAS))

            scr_act = const.tile([1, 1], F32, tag="scr_act")
            acol = nc.scalar.activation(
                scr_act, ebias[0:1, :], mybir.ActivationFunctionType.Copy
            )
            add_dep_helper(acol.ins, eb_inst.ins, True, "act bias barrier")
            last["a"] = acol.ins

            for k, ld in enumerate([b_load, xq_load]):
                scr_k = const.tile([1, 1], F32, tag=f"scr{k}", name=f"scr{k}")
                dcol = nc.vector.memset(scr_k, 0.0)
                add_dep_helper(dcol.ins, ld.ins, True, "dve input barrier")
                chain("v", dcol)

            # ---- HAM warmup: keep the PE active during the initial DMA
            # wait so the clock gate is at 8/8 when the real work starts
            warm = const.tile([128, 2, 512], FP8, tag="warm")
            chain("v", nc.vector.memset(warm, 0.0))
            warm_ps = pso.tile([1, 512], F32, tag="misc", name="warm_ps")
            for _ in range(30):
                chain("p", nc.tensor.matmul(
                    warm_ps, lhsT=ones8[:, :, 0:1], rhs=warm,
                    perf_mode=DR, start=True, stop=True,
                ))

            ps_col = pso.tile([1, 1], F32, tag="misc", name="ps_col")
            probe = bb[0:1, 0, 0:1]

            def pe_barrier(ld):
                col = nc.tensor.matmul(ps_col, lhsT=probe, rhs=probe)
                add_dep_helper(col.ins, ld.ins, True, "pe input barrier")
                chain("p", col)

            pe_barrier(w_load)
            pe_barrier(xk_loads[0])

            def mm(*args, **kwargs):
                return chain("p", nc.tensor.matmul(*args, **kwargs))

            def dve(fn, *args, **kwargs):
                return chain("v", fn(*args, **kwargs))

            def act(*args, **kwargs):
                return chain("a", nc.scalar.activation(*args, **kwargs))

            # ---- projections, gated per xk chunk (one 512-col group per
            # chunk); eviction + bias on the DVE so the Scalar engine is
            # reserved for the softmax exp stream (the attention pipeline
            # overlaps the later projection groups) ----
            def project_g(w_idx, dst, g, bias_col):
                # theta evictions ride the Scalar engine (idle until the
                # exp stream starts); phi evictions ride the DVE, so the
                # two eviction streams run concurrently
                for co in range(CT):
                    ps_pj = psf.tile(
                        [128, 512], F32, tag=f"f{co % 2}", name="ps_pj"
                    )
                    mm(
                        ps_pj,
                        lhsT=ws_sb[:, w_idx, 0:2, co * 128 : (co + 1) * 128],
                        rhs=xk_sb[:, g, 0:2, :],
                        perf_mode=DR,
                        start=True,
                        stop=True,
                    )
                    if w_idx == 0:
                        act(
                            dst[:, co, g * 512 : (g + 1) * 512],
                            ps_pj,
                            mybir.ActivationFunctionType.Identity,
                            bias=bb[:, bias_col, co : co + 1],
                        )
                    else:
                        dve(
                            nc.vector.tensor_scalar_add,
                            dst[:, co, g * 512 : (g + 1) * 512],
                            ps_pj,
                            bb[:, bias_col, co : co + 1],
                        )

            for g in range(NCH):
                if g > 0:
                    pe_barrier(xk_loads[g])
                if g < PG:
                    project_g(0, th_sb, g, 0)
                project_g(1, ph_sb, g, 1)
            pe_barrier(xt_loads[0])
            pe_barrier(wc_load)

            # ---- attention; per-group finalization is deferred into the
            # next group so the normalize chain (PE ones-reduce -> DVE
            # reciprocal -> PE broadcast -> DVE muls) overlaps PE work
            deferred = [None]

            def finalize_bc(pg):
                psl = slice(pg * 512, (pg + 1) * 512)
                ps_fs = deferred[0][1]
                # shares the "o" bank with the out-conv PSUM: the bc write
                # and its copy-out strictly precede the conv matmuls, so the
                # pool rotation sequences them without a live-range overlap
                # (the "misc" bank stays dedicated to ps_sum, whose
                # accumulation group spans most of the query-group loop).
                ps_bc = pso.tile([128, 512], F32, tag="o", name="ps_bc")
                mm(ps_bc, lhsT=ones_col, rhs=rc_ring[:, pg, :])
                bc_sb = bc_ring[:, pg, :]
                dve(nc.vector.tensor_copy, bc_sb, ps_bc)
                for ci in range(CT):
                    dve(
                        nc.vector.tensor_mul, f_sb[:, ci, psl], ps_fs[ci], bc_sb
                    )

            def finalize_conv(pg, last=False):
                psl = slice(pg * 512, (pg + 1) * 512)
                for co in range(CT):
                    if last and co == 1:
                        # the scores PSUM is idle during the final drain;
                        # borrowing a slot lets co1's conv overlap co0's
                        # eviction instead of waiting for the "o" bank
                        ps_o_pair = pss.tile(
                            [128, 2, 512], F32, tag="s", bufs=2, name="ps_o_pair"
                        )
                        ps_o = ps_o_pair[:, 0]
                    else:
                        ps_o = pso.tile([128, 512], F32, tag="o", name="ps_o")
                    for ci in range(CT):
                        mm(
                            ps_o,
                            lhsT=wc_sb[:, ci, co * 128 : (co + 1) * 128],
                            rhs=f_sb[:, ci, psl],
                            start=(ci == 0),
                            stop=(ci == CT - 1),
                        )
                    dve(
                        nc.vector.scalar_tensor_tensor,
                        out=o_ring[:, co, pg, :],
                        in0=ps_o,
                        scalar=bb[:, 2, co : co + 1],
                        in1=xq_sb[:, co, psl],
                        op0=add,
                        op1=add,
                    )
                deferred[0] = None

            for pg in range(PG):
                psl = slice(pg * 512, (pg + 1) * 512)
                ps_f = [
                    psf.tile([128, 512], F32, tag=f"f{ci}", name=f"ps_f{ci}")
                    for ci in range(CT)
                ]
                ps_sum = pso.tile([1, 512], F32, tag="misc", name="ps_sum")

                def denom(qpp, e_p):
                    mm(
                        ps_sum,
                        lhsT=ones8[:, :, 0:1],
                        rhs=e_p[:, 0:2, :],
                        perf_mode=DR,
                        start=(qpp == 0),
                        stop=(qpp == NP - 1),
                    )

                # software pipeline: PV runs one exp-pair behind scores so
                # the PE streams scores(k+1) while ACT computes exp(k)
                for qp in range(NP + 1):
                    if pg == 0 and qp == 8:
                        # second xt half gates the PV of pairs >= 8
                        pe_barrier(xt_loads[1])
                    if qp < NP:
                        ps_s = pss.tile([128, 2, 512], F32, tag="s", bufs=2)
                        for sub in range(2):
                            qt = qp * 2 + sub
                            mm(
                                ps_s[:, sub],
                                lhsT=ph_sb[:, 0:2, qt * 128 : (qt + 1) * 128],
                                rhs=th_sb[:, 0:2, psl],
                                perf_mode=DR,
                                start=True,
                                stop=True,
                            )
                        act(
                            e_ring[:, qp % 4], ps_s,
                            mybir.ActivationFunctionType.Exp,
                            bias=ebias, scale=scale,
                        )
                    if qp == 1 and deferred[0] is not None:
                        finalize_bc(pg - 1)
                    if qp == 2 and deferred[0] is not None:
                        finalize_conv(pg - 1)
                    if qp >= 1:
                        qpp = qp - 1
                        e_p = e_ring[:, qpp % 4]
                        # softmax denominators: ones-row DoubleRow matmuls
                        # accumulating into ps_sum.  The qpp==0 pair is
                        # deferred to qp==2 so the misc PSUM bank has been
                        # released by the previous group's broadcast.
                        if qpp == 1:
                            denom(0, e_ring[:, 0])
                            denom(1, e_p)
                        elif qpp >= 2:
                            denom(qpp, e_p)
                        for ci in range(CT):
                            mm(
                                ps_f[ci],
                                lhsT=xt_sb[
                                    :, 2 * qpp : 2 * qpp + 2,
                                    ci * 128 : (ci + 1) * 128,
                                ],
                                rhs=e_p[:, 0:2, :],
                                perf_mode=DR,
                                start=(qpp == 0),
                                stop=(qpp == NP - 1),
                            )

                # 1/sum on DVE (custom op; ~18 correct bits, no ACT table).
                # Emitted via _custom_dve so the output can be declared
                # float32r (same bits as fp32) for the broadcast matmul.
                from concourse.dve_ops import (
                    RECIP_APPROX_FAST_CONSTS,
                    RECIPROCAL_APPROX_FAST,
                )

                rc = RECIP_APPROX_FAST_CONSTS
                dve(
                    nc.vector._custom_dve,
                    RECIPROCAL_APPROX_FAST,
                    out=rc_ring[:, pg, :],
                    in0=ps_sum,
                    s0=rc["s0"],
                    s1=rc["s1"],
                    imm2=rc["imm2"],
                )
                deferred[0] = (pg, ps_f)

            finalize_bc(PG - 1)
            finalize_conv(PG - 1, last=True)

            # ---- output stores: two halves per channel tile with
            # 2KB-contiguous descriptors; the first half (groups 0-1) can
            # fire as soon as those groups are evicted, the second half is
            # the only post-compute tail ----
            for h in range(2):
                for co in range(CT):
                    nc.sync.dma_start(
                        out=out[
                            co * 128 : (co + 1) * 128,
                            h * (P // 2) : (h + 1) * (P // 2),
                        ],
                        in_=o_ring[:, co, 2 * h : 2 * h + 2, :],
                    )
    nc.compile()
    return nc


_PROGRAM_CACHE = {}


def _get_program(mm_dt=mybir.dt.float16):
    key = str(mm_dt)
    if key not in _PROGRAM_CACHE:
        _PROGRAM_CACHE[key] = build_program(P_CORE, HW, C, mm_dt)
    return _PROGRAM_CACHE[key]


def make_in_maps(x, theta_w, theta_b, phi_w, phi_b, conv1_w, conv1_b,
                 mm_np=np.float16):
    """Host-side sharding / layout prep (pure data movement, no math)."""
    fp8_np = mybir.dt.np(FP8)
    # theta/phi weights fp8, pre-scaled x16 (exp scale folds it back out)
    # and pre-interleaved partition-major [128, 2, CT, Cout]
    wqk = (
        np.stack(
            [
                np.asarray(theta_w, np.float32).T * 16.0,
                np.asarray(phi_w, np.float32).T * 16.0,
            ]
        )
        .reshape(2, C // 128, 128, C)
        .transpose(2, 0, 1, 3)
    )
    wqk = np.ascontiguousarray(wqk).astype(fp8_np)
    # conv weight fp16, [128, CT, Cout]
    wc = (
        np.asarray(conv1_w, np.float32)
        .T.reshape(C // 128, 128, C)
        .transpose(1, 0, 2)
    )
    wc = np.ascontiguousarray(wc).astype(mm_np)
    bcat = np.ascontiguousarray(
        np.stack(
            [
                np.asarray(theta_b, np.float32) * 16.0,
                np.asarray(phi_b, np.float32) * 16.0,
                np.asarray(conv1_b, np.float32),
            ]
        )
    )
    xf = np.asarray(x, np.float32).reshape(N, C, HW)
    in_maps = []
    for core in range(NCORES):
        n, half = divmod(core, CORES_PER_N)
        off = half * P_CORE
        xk_i = np.ascontiguousarray(np.roll(xf[n], -off, axis=1))
        xt_i = xk_i.T.reshape(HW // 128, 128, C).swapaxes(0, 1)
        # xk pre-interleaved partition-major by 512-col chunk:
        # [128, NCH, CT, 512] with [p, ch, a, q] = xk[a*128+p, ch*512+q]
        xk_p = (
            xk_i.reshape(C // 128, 128, HW // 512, 512)
            .transpose(1, 2, 0, 3)
        )
        # fp16 residual copy of the query columns, [128, CT, P]
        xq_p = xk_i[:, :P_CORE].reshape(C // 128, 128, P_CORE).transpose(1, 0, 2)
        in_maps.append(
            {
                "xk": np.ascontiguousarray(xk_p).astype(fp8_np),
                "xt": np.ascontiguousarray(xt_i).astype(fp8_np),
                "xq": np.ascontiguousarray(xq_p).astype(mm_np),
                "wqk": wqk,
                "wc": wc,
                "bcat": bcat,
            }
        )
    return in_maps


def assemble_output(results):
    y = np.empty((N, C, HW), np.float32)
    for core in range(NCORES):
        n, half = divmod(core, CORES_PER_N)
        off = half * P_CORE
        y[n][:, off : off + P_CORE] = results[core]["out"].astype(np.float32)
    return y.reshape(N, C, H, W)


def kernel(x, theta_w, theta_b, phi_w, phi_b, conv1_w, conv1_b,
           mm_dt=None, **run_kwargs):
    if mm_dt is None:
        mm_dt = mybir.dt.float16
    nc = _get_program(mm_dt)
    in_maps = make_in_maps(
        x, theta_w, theta_b, phi_w, phi_b, conv1_w, conv1_b,
        mm_np=mybir.dt.np(mm_dt),
    )
    res = run_bass_kernel_spmd(nc, in_maps, list(range(NCORES)), **run_kwargs)
    out = assemble_output(res.results)
    kernel.last_results = res
    return out
